# revision 10
# baseline (speedup 1.0000x reference)
"""Causal self-attention (B=8, T=1024, C=768, NH=12) on 8 TRN2 NeuronCores.

Strategy: pure batch data-parallel — core b computes batch element b end to
end (no collectives).

PE cost model (instruction_cost_v2): matmul streaming cost = N output
cols x 0.417ns; K<=128 is the parallel dim.  K=64 matmuls with lhsT at
partition 0 vs 64 get tile_position (0,0)/(64,0) automatically and run
CONCURRENTLY (2x row tiling) — but only if nothing in between switches
the array back to 128-row mode (mode switch = drain).

Engines are strict-FIFO, so a long-latency wait at the head of a queue
(e.g. a DVE multiply waiting on a DMA round-trip) blocks everything
behind it.  The softmax divide chain is therefore DEFERRED by one
attention block: block k emits only psum->sbuf evac + the denominator
gather DMA; the recip/broadcast/multiply for block k are emitted after
block k+1's evac, by which point their DMA inputs have landed.

Per-core dataflow (everything kept "transposed", i.e. [feature, time]):
  xT [C, T]                                  (host pre-transposes x[b])
  qkT[j, t] = Wqk[:, j].T x  (bf16)          psum evac on Act (+bias)
  v  [t, j] = x Wv           (fp32r)         AV-friendly layout, augmented
                                             with a ones column per head
  attT[tk, tq] = kT.T @ qT   per head pair   K=64 pair-tiled matmuls; the
                                             two parities land at psum cols
                                             [cs,TQ) and [TQ,2TQ-cs) (par1
                                             shifted left by cs: no gap)
  causal diag mask: two K=64 triangular bf16 matmuls per (diag,par) add
                                             -1e30 in the SAME 64-row mode
  expT = exp(scale * attT)   one Act op over the contiguous [cs, 2TQ-cs)
  out_aug = [v | 1 | junk].T @ expT          row 64 = softmax denominator
  rawT[j, t] = out_aug[d] * (1/denom)        denom broadcast via DMA (bf16)
  yT[e, t] = Wp.T @ rawT + bp'               bp' = Wp.T bv + bp (host).
                                             The t2=1 half is split into a
                                             5-chunk partial + a final
                                             single matmul so the tail
                                             doesn't serialize behind the
                                             last divide chain.
"""

import os
import sys

import numpy as np

for _p in ("/opt/trn_rl_repo", "/root/.axon_site/_ro/trn_rl_repo"):
    if os.path.isdir(_p) and _p not in sys.path:
        sys.path.insert(0, _p)

import ml_dtypes

import concourse.bacc as bacc
import concourse.mybir as mybir
import concourse.tile as tile
from concourse.bass import ts
from concourse.bass_utils import run_bass_kernel_spmd

B, T, C = 8, 1024, 768
NH, HD = 12, 64
P = 128
NCORES = 8
CC = C // P            # 6 contraction chunks over C
JQK = 2 * C // P       # 12 output chunks for q|k
EC = C // P            # 6 output chunks for the projection
TQ = 512               # moving-dim tile (max psum bank width)
NTQ = T // TQ          # 2
NTK = T // P           # 8 key chunks
G = NH // 2            # 6 head pairs (two 64-wide heads per 128 partitions)
VW = 2 * HD + 2        # 130: per-pair v layout [d_even(64), 1, d_odd(64), 1]
VPAD = 63              # tail pad so the odd lhsT can always grab 128 cols
JV = 384               # v output tile width (3 head pairs)
SCALE = 1.0 / float(np.sqrt(HD))
F32 = mybir.dt.float32
F32R = mybir.dt.float32r
BF16 = mybir.dt.bfloat16
AF = mybir.ActivationFunctionType
ADD = mybir.AluOpType.add
MUL = mybir.AluOpType.mult

_CACHE = {}


def _build():
    if "nc" in _CACHE:
        return _CACHE["nc"]

    nc = bacc.Bacc("TRN2", target_bir_lowering=False, debug=False)

    xT = nc.dram_tensor("xT", [C, T], F32R, kind="ExternalInput")
    wqk = nc.dram_tensor("wqk", [C, 2 * C], F32R, kind="ExternalInput")
    wv = nc.dram_tensor("wv", [C, C], F32R, kind="ExternalInput")
    wp = nc.dram_tensor("wp", [C, C], F32R, kind="ExternalInput")
    bqk = nc.dram_tensor("bqk", [P, JQK], F32, kind="ExternalInput")
    bp = nc.dram_tensor("bp", [P, EC], F32, kind="ExternalInput")
    tri = nc.dram_tensor("tri", [P, 2, P], BF16, kind="ExternalInput")
    id64 = nc.dram_tensor("id64", [P, HD], BF16, kind="ExternalInput")
    yT = nc.dram_tensor("yT", [C, T], F32, kind="ExternalOutput")

    xT_r = xT[:].rearrange("(o p) t -> p o t", p=P)
    wqk_r = wqk[:].rearrange("(o p) j -> p o j", p=P)
    wv_r = wv[:].rearrange("(o p) j -> p o j", p=P)
    wp_r = wp[:].rearrange("(o p) e -> p o e", p=P)
    yT_r = yT[:].rearrange("(o p) t -> p o t", p=P)

    with tile.TileContext(nc) as tc:
        with (
            tc.tile_pool(name="const", bufs=1) as constp,
            tc.tile_pool(name="xt", bufs=6) as xtp,
            tc.tile_pool(name="wqk", bufs=4) as wqkp,
            tc.tile_pool(name="wv", bufs=1) as wvp,
            tc.tile_pool(name="qkt", bufs=1) as qkTp,
            tc.tile_pool(name="vaug", bufs=1) as vap,
            tc.tile_pool(name="raw", bufs=1) as rawp,
            tc.tile_pool(name="wp", bufs=6) as wpp,
            tc.tile_pool(name="exp", bufs=3) as expp,
            tc.tile_pool(name="rr", bufs=3) as rrp,
            tc.tile_pool(name="yt", bufs=3) as ytp,
            tc.tile_pool(name="yta", bufs=6) as ytap,
            tc.tile_pool(name="psA", bufs=2, space="PSUM") as psA,
            tc.tile_pool(name="psB", bufs=2, space="PSUM") as psB,
        ):
            # ---- resident tensors (DMA issue order matters: x halves
            # first so qkT can start, then the g=0 weights, then wv) ----
            xts = []
            for cc in range(CC):
                xt_t = xtp.tile([P, T], F32R, tag="xt", name=f"xt{cc}")
                nc.sync.dma_start(xt_t[:, 0:TQ], xT_r[:, cc, 0:TQ])
                xts.append(xt_t)
            for cc in range(CC):
                nc.sync.dma_start(xts[cc][:, TQ:T], xT_r[:, cc, TQ:T])

            bqk_sb = constp.tile([P, JQK], F32)
            nc.sync.dma_start(bqk_sb[:], bqk[:])
            bp_sb = constp.tile([P, EC], F32)
            nc.sync.dma_start(bp_sb[:], bp[:])
            tri_sb = constp.tile([P, 2, P], BF16)
            nc.sync.dma_start(tri_sb[:], tri[:])
            id64_sb = constp.tile([P, HD], BF16)
            nc.sync.dma_start(id64_sb[:], id64[:])

            qkT_sb = qkTp.tile([P, JQK, T], BF16)
            v_sb = vap.tile([P, NTK, G * VW + VPAD], BF16)
            v4 = v_sb[:, :, : G * VW].rearrange("p n (g w) -> p n g w", w=VW)
            rawT = rawp.tile([P, CC, T], F32R)

            # ---- v[t, j] = x Wv, interleaved per head pair ------------
            # ones columns feed the softmax-denominator trick; tail pad is
            # junk-read by the odd head's M=128 lhsT (rows 65+ of its psum)
            onec = constp.tile([P, 1], F32)
            nc.vector.memset(onec[:], 1.0)
            zeroc = constp.tile([P, 1], F32)
            nc.vector.memset(zeroc[:], 0.0)
            ones_src = onec[:, None, None, :].to_broadcast([P, NTK, G, 1])
            nc.any.tensor_copy(v4[:, :, :, HD : HD + 1], ones_src)
            nc.any.tensor_copy(v4[:, :, :, VW - 1 : VW], ones_src)
            nc.any.tensor_copy(
                v_sb[:, :, G * VW :],
                zeroc[:, None, :].to_broadcast([P, NTK, VPAD]),
            )

            wv_sb = wvp.tile([P, CC, C], F32R)

            def v_part(tcs):
                for tc_i in tcs:
                    for jn in range(C // JV):
                        ps = psB.tile([P, TQ], F32, tag="mm", name="psv")
                        for cc in range(CC):
                            nc.tensor.matmul(
                                ps[:, :JV],
                                xts[cc][:, ts(tc_i, P)],
                                wv_sb[:, cc, ts(jn, JV)],
                                start=(cc == 0),
                                stop=(cc == CC - 1),
                            )
                        g0 = jn * (JV // P)  # 3 head pairs per 384 cols
                        srcv = ps[:, :JV].rearrange(
                            "p (g h d) -> p g h d", h=2, d=HD
                        )
                        # psum -> interleaved sbuf layout on the Act engine
                        # (DVE stays free for the divide chains); the even
                        # and odd head halves are one strided 4D AP
                        dstv = v4[:, tc_i, g0 : g0 + 3, :].rearrange(
                            "p g (h w) -> p g h w", h=2
                        )[:, :, :, :HD]
                        nc.scalar.activation(dstv, srcv, AF.Identity)

            def load_wt(jc):
                wt = wqkp.tile([P, CC, P], F32R, tag="wqk", name="wt")
                nc.sync.dma_start(wt[:], wqk_r[:, :, ts(jc, P)])
                return wt

            def qkt_chunk(jc, wt):
                for t2 in range(NTQ):
                    ps = psB.tile([P, TQ], F32, tag="mm", name="psq")
                    for cc in range(CC):
                        nc.tensor.matmul(
                            ps[:],
                            wt[:, cc, :],
                            xts[cc][:, ts(t2, TQ)],
                            start=(cc == 0),
                            stop=(cc == CC - 1),
                        )
                    nc.vector.tensor_scalar_add(
                        qkT_sb[:, jc, ts(t2, TQ)],
                        ps[:],
                        bqk_sb[:, jc : jc + 1],
                    )

            def attn_block(g, t2):
                """Emit QK/exp/AV + psum evac + denominator gather for one
                block.  Returns a closure that finishes the divide (recip,
                broadcast, multiply into rawT) — call it one block later."""
                jq, jk = g, G + g
                hi = 4 * (t2 + 1)  # causal: key chunks 0..hi-1
                avs = []
                for par in (0, 1):
                    av = psB.tile([P, TQ], F32, tag="av", name=f"av{par}")
                    avs.append(av)
                for g0 in range(0, hi, 2):
                    grp = [tkc for tkc in (g0, g0 + 1) if tkc < hi]
                    pas, css = {}, {}
                    # all QK + diag-mask matmuls of the group first: they
                    # are K=64 and pair-tile as (0,0)/(64,0); keeping them
                    # adjacent avoids PE array mode switches
                    for tkc in grp:
                        csr = tkc * P - t2 * TQ  # diag block start col
                        cs = max(0, csr)
                        pa = psA.tile([P, 2 * TQ], F32, tag="pa", name="pa")
                        pas[tkc], css[tkc] = pa, cs
                        for par in (0, 1):
                            qrow = HD * par
                            lo = cs if par == 0 else TQ
                            nc.tensor.matmul(
                                pa[:, lo : lo + TQ - cs],
                                qkT_sb[qrow : qrow + HD, jk, ts(tkc, P)],
                                qkT_sb[
                                    qrow : qrow + HD,
                                    jq,
                                    t2 * TQ + cs : (t2 + 1) * TQ,
                                ],
                                start=True,
                                stop=(csr < 0),
                            )
                        if csr >= 0:
                            # add -1e30 above the diagonal with two K=64
                            # triangular matmuls (same row-tiled mode):
                            # pa[tk, lo+j] += tri[j, tk]
                            for par in (0, 1):
                                qrow = HD * par
                                lo = cs if par == 0 else TQ
                                for half in (0, 1):
                                    nc.tensor.matmul(
                                        pa[
                                            :,
                                            lo + HD * half : lo + HD * (half + 1),
                                        ],
                                        tri_sb[qrow : qrow + HD, half, :],
                                        id64_sb[qrow : qrow + HD, :],
                                        start=False,
                                        stop=(half == 1),
                                    )
                    for tkc in grp:
                        pa, cs = pas[tkc], css[tkc]
                        e = expp.tile([P, 2 * TQ], BF16, tag="exp", name="e")
                        nc.scalar.activation(
                            e[:, cs : 2 * TQ - cs],
                            pa[:, cs : 2 * TQ - cs],
                            AF.Exp,
                            scale=SCALE,
                        )
                        for par in (0, 1):
                            lo = cs if par == 0 else TQ
                            vlo = g * VW + (HD + 1) * par
                            nc.tensor.matmul(
                                avs[par][:, cs:],
                                v_sb[:, tkc, vlo : vlo + P],
                                e[:, lo : lo + TQ - cs],
                                start=(tkc == 0),
                                stop=(tkc == hi - 1),
                            )
                # evacuate out_aug to SBUF right away in bf16 (frees the
                # psum banks fast) and gather the two denominator rows to
                # [128, 8] via DMA so the reciprocal runs on all lanes.
                asb = rrp.tile([65, 2, TQ], BF16, tag="avsb", name="asb")
                for par in (0, 1):
                    nc.vector.tensor_scalar_add(
                        asb[:, par, :], avs[par][0:65, :], 0.0
                    )
                rd = rrp.tile([P, 8], BF16, tag="rd", name="rd")
                nc.sync.dma_start(rd[:], asb[64:65, :, :])

                def finish():
                    rd2 = rrp.tile([P, 8], BF16, tag="rd2", name="rd2")
                    with nc.allow_low_precision(
                        reason="softmax denom reciprocal in bf16; rel err "
                        "~4e-3 well inside the 2e-2 budget"
                    ):
                        nc.vector.reciprocal(rd2[:], rd[:])
                    rro = rrp.tile([1, 2, TQ], BF16, tag="rro", name="rro")
                    nc.sync.dma_start(rro[0:1, :, :], rd2[:])
                    rrs = []
                    for par in (0, 1):
                        rr = rrp.tile([64, TQ], BF16, tag=f"rr{par}", name="rr")
                        nc.sync.dma_start(
                            rr[:],
                            rro[0:1, par, None, :].to_broadcast([1, 64, TQ]),
                        )
                        rrs.append(rr)
                    nc.vector.tensor_tensor(
                        rawT[0:64, g, ts(t2, TQ)],
                        asb[0:64, 0, :],
                        rrs[0][:],
                        MUL,
                    )
                    tmp = rrp.tile([64, TQ], F32R, tag="otmp", name="tmp")
                    nc.vector.tensor_tensor(
                        tmp[:], asb[0:64, 1, :], rrs[1][:], MUL
                    )
                    nc.sync.dma_start(rawT[64:128, g, ts(t2, TQ)], tmp[:])

                return finish

            def load_wpt(ec):
                wpt = wpp.tile([P, CC, P], F32R, tag="wp", name="wpt")
                nc.sync.dma_start(wpt[:], wp_r[:, :, ts(ec, P)])
                return wpt

            # The projection is split per output half into a 5-chunk
            # partial plus a single final matmul on the last head pair, so
            # no proj work ever waits on the most recent divide chain.
            def proj_partial(t2, ec, wpt):
                ps = psB.tile([P, TQ], F32, tag="mm", name="pspa")
                for jc in range(CC - 1):
                    nc.tensor.matmul(
                        ps[:],
                        wpt[:, jc, :],
                        rawT[:, jc, ts(t2, TQ)],
                        start=(jc == 0),
                        stop=(jc == CC - 2),
                    )
                yta = ytap.tile([P, TQ], F32, tag="yta", name="yta")
                nc.vector.tensor_scalar_add(yta[:], ps[:], 0.0)
                return yta

            def proj_final(t2, ec, wpt, yta):
                ps = psB.tile([P, TQ], F32, tag="mm", name="pspb")
                nc.tensor.matmul(
                    ps[:],
                    wpt[:, CC - 1, :],
                    rawT[:, CC - 1, ts(t2, TQ)],
                    start=True,
                    stop=True,
                )
                yt = ytp.tile([P, TQ], F32, tag="yt", name="yt")
                nc.vector.scalar_tensor_tensor(
                    yt[:], ps[:], bp_sb[:, ec : ec + 1], yta[:], ADD, ADD
                )
                nc.sync.dma_start(yT_r[:, ec, ts(t2, TQ)], yt[:])

            # ---- emission schedule ------------------------------------
            # startup: two qkT pairs back to back keep the PE streaming
            # while wv lands; v then fills tkc 0-3 ahead of attention.
            wts = (load_wt(0), load_wt(G))
            nxt = (load_wt(1), load_wt(G + 1))
            nc.sync.dma_start(wv_sb[:], wv_r)
            qkt_chunk(0, wts[0])
            qkt_chunk(G, wts[1])
            qkt_chunk(1, nxt[0])
            qkt_chunk(G + 1, nxt[1])
            v_part(range(0, 4))
            nxt = (load_wt(2), load_wt(G + 2))
            fin = attn_block(0, 0)
            v_part(range(4, 6))
            wpts = []
            for g in range(1, G):
                if g >= 2:
                    wts = nxt
                    qkt_chunk(g, wts[0])
                    qkt_chunk(G + g, wts[1])
                    if g < G - 1:
                        nxt = (load_wt(g + 1), load_wt(G + g + 1))
                    else:
                        wpts = [load_wpt(ec) for ec in range(EC)]
                if g == 1:
                    v_part(range(6, 8))
                prev, fin = fin, attn_block(g, 0)
                prev()
            ytas0 = [None] * EC
            for g in range(G):
                prev, fin = fin, attn_block(g, 1)
                prev()
                if g in (1, 2):
                    for ec in (3 * g - 3, 3 * g - 2, 3 * g - 1):
                        ytas0[ec] = proj_partial(0, ec, wpts[ec])
                elif g >= 3:
                    for ec in (2 * g - 6, 2 * g - 5):
                        proj_final(0, ec, wpts[ec], ytas0[ec])
            fin()  # divide for the last block (5, 1)
            ytas1 = [proj_partial(1, ec, wpts[ec]) for ec in range(EC)]
            for ec in range(EC):
                proj_final(1, ec, wpts[ec], ytas1[ec])

    nc.compile()
    _CACHE["nc"] = nc
    return nc


def _round_fp32r(a):
    """Round fp32 to fp32r (11-bit mantissa) the way the PE expects."""
    u = np.ascontiguousarray(a, dtype=np.float32).view(np.uint32)
    u = ((u.astype(np.uint64) + 0x800) & 0xFFFFF000).astype(np.uint32)
    return u.view(np.float32)


def make_in_maps(x, w_attn, b_attn, w_proj, b_proj):
    x = np.ascontiguousarray(np.asarray(x, dtype=np.float32))
    w_attn = np.ascontiguousarray(np.asarray(w_attn, dtype=np.float32))
    b_attn = np.ascontiguousarray(np.asarray(b_attn, dtype=np.float32))
    w_proj = np.ascontiguousarray(np.asarray(w_proj, dtype=np.float32))
    b_proj = np.ascontiguousarray(np.asarray(b_proj, dtype=np.float32))

    bf = ml_dtypes.bfloat16
    wqk = _round_fp32r(w_attn[:, : 2 * C])
    wv = _round_fp32r(w_attn[:, 2 * C :])
    w_proj_r = _round_fp32r(w_proj)
    bqk = np.ascontiguousarray(b_attn[: 2 * C].reshape(JQK, P).T)
    # the v bias folds into the projection bias: y = Wp.T (raw + bv) + bp
    bv = b_attn[2 * C :].astype(np.float64)
    bp_eff = (w_proj.astype(np.float64).T @ bv + b_proj).astype(np.float32)
    bp = np.ascontiguousarray(bp_eff.reshape(EC, P).T)
    # tri[qrow+r, 0, tk] masks j=r    < tk; tri[qrow+r, 1, tk] masks 64+r < tk
    tri = np.zeros((P, 2, P), dtype=bf)
    tk = np.arange(P)[None, :]
    for qrow in (0, HD):
        r = np.arange(HD)[:, None]
        tri[qrow : qrow + HD, 0, :] = np.where(r < tk, -1e30, 0.0).astype(bf)
        tri[qrow : qrow + HD, 1, :] = np.where(r + HD < tk, -1e30, 0.0).astype(
            bf
        )
    id64 = np.zeros((P, HD), dtype=bf)
    for qrow in (0, HD):
        id64[qrow : qrow + HD, :] = np.eye(HD, dtype=bf)

    shared = {
        "wqk": wqk,
        "wv": wv,
        "wp": w_proj_r,
        "bqk": bqk,
        "bp": bp,
        "tri": tri,
        "id64": id64,
    }
    return [
        {"xT": _round_fp32r(x[b].T), **shared} for b in range(NCORES)
    ]


def kernel(**inputs):
    nc = _build()
    in_maps = make_in_maps(
        inputs["x"],
        inputs["w_attn"],
        inputs["b_attn"],
        inputs["w_proj"],
        inputs["b_proj"],
    )
    res = run_bass_kernel_spmd(nc, in_maps, list(range(NCORES)))
    out = np.stack(
        [np.ascontiguousarray(res.results[b]["yT"].T) for b in range(NCORES)]
    )
    return out.astype(np.float32)


# revision 13
# speedup vs baseline: 1.0009x; 1.0009x over previous
"""Causal self-attention (B=8, T=1024, C=768, NH=12) on 8 TRN2 NeuronCores.

Strategy: pure batch data-parallel — core b computes batch element b end to
end (no collectives).

PE cost model (instruction_cost_v2): matmul streaming cost = N output
cols x 0.417ns; K<=128 is the parallel dim.  K=64 matmuls with lhsT at
partition 0 vs 64 get tile_position (0,0)/(64,0) automatically and run
CONCURRENTLY (2x row tiling) — but only if nothing in between switches
the array back to 128-row mode (mode switch = drain).

Engines are strict-FIFO, so a long-latency wait at the head of a queue
(e.g. a DVE multiply waiting on a DMA round-trip) blocks everything
behind it.  The softmax divide chain is therefore DEFERRED by one
attention block: block k emits only psum->sbuf evac + the denominator
gather DMA; the recip/broadcast/multiply for block k are emitted after
block k+1's evac, by which point their DMA inputs have landed.

Per-core dataflow (everything kept "transposed", i.e. [feature, time]):
  xT [C, T]                                  (host pre-transposes x[b])
  qkT[j, t] = Wqk[:, j].T x  (bf16)          psum evac on Act (+bias)
  v  [t, j] = x Wv           (fp32r)         AV-friendly layout, augmented
                                             with a ones column per head
  attT[tk, tq] = kT.T @ qT   per head pair   K=64 pair-tiled matmuls; the
                                             two parities land at psum cols
                                             [cs,TQ) and [TQ,2TQ-cs) (par1
                                             shifted left by cs: no gap)
  causal diag mask: two K=64 triangular bf16 matmuls per (diag,par) add
                                             -1e30 in the SAME 64-row mode
  expT = exp(scale * attT)   one Act op over the contiguous [cs, 2TQ-cs)
  out_aug = [v | 1 | junk].T @ expT          row 64 = softmax denominator
  rawT[j, t] = out_aug[d] * (1/denom)        denom broadcast via DMA (bf16)
  yT[e, t] = Wp.T @ rawT + bp'               bp' = Wp.T bv + bp (host).
                                             The t2=1 half is split into a
                                             5-chunk partial + a final
                                             single matmul so the tail
                                             doesn't serialize behind the
                                             last divide chain.
"""

import os
import sys

import numpy as np

for _p in ("/opt/trn_rl_repo", "/root/.axon_site/_ro/trn_rl_repo"):
    if os.path.isdir(_p) and _p not in sys.path:
        sys.path.insert(0, _p)

import ml_dtypes

import concourse.bacc as bacc
import concourse.mybir as mybir
import concourse.tile as tile
from concourse.bass import ts
from concourse.bass_utils import run_bass_kernel_spmd

B, T, C = 8, 1024, 768
NH, HD = 12, 64
P = 128
NCORES = 8
CC = C // P            # 6 contraction chunks over C
JQK = 2 * C // P       # 12 output chunks for q|k
EC = C // P            # 6 output chunks for the projection
TQ = 512               # moving-dim tile (max psum bank width)
NTQ = T // TQ          # 2
NTK = T // P           # 8 key chunks
G = NH // 2            # 6 head pairs (two 64-wide heads per 128 partitions)
VW = 2 * HD + 2        # 130: per-pair v layout [d_even(64), 1, d_odd(64), 1]
VPAD = 63              # tail pad so the odd lhsT can always grab 128 cols
JV = 384               # v output tile width (3 head pairs)
SCALE = 1.0 / float(np.sqrt(HD))
F32 = mybir.dt.float32
F32R = mybir.dt.float32r
BF16 = mybir.dt.bfloat16
AF = mybir.ActivationFunctionType
ADD = mybir.AluOpType.add
MUL = mybir.AluOpType.mult

_CACHE = {}


def _build():
    if "nc" in _CACHE:
        return _CACHE["nc"]

    nc = bacc.Bacc("TRN2", target_bir_lowering=False, debug=False)

    xT = nc.dram_tensor("xT", [C, T], F32R, kind="ExternalInput")
    wqk = nc.dram_tensor("wqk", [C, 2 * C], F32R, kind="ExternalInput")
    wv = nc.dram_tensor("wv", [C, C], F32R, kind="ExternalInput")
    wp = nc.dram_tensor("wp", [C, C], F32R, kind="ExternalInput")
    bqk = nc.dram_tensor("bqk", [P, JQK], F32, kind="ExternalInput")
    bp = nc.dram_tensor("bp", [P, EC], F32, kind="ExternalInput")
    tri = nc.dram_tensor("tri", [P, 2, P], BF16, kind="ExternalInput")
    id64 = nc.dram_tensor("id64", [P, HD], BF16, kind="ExternalInput")
    yT = nc.dram_tensor("yT", [C, T], F32, kind="ExternalOutput")

    xT_r = xT[:].rearrange("(o p) t -> p o t", p=P)
    wqk_r = wqk[:].rearrange("(o p) j -> p o j", p=P)
    wv_r = wv[:].rearrange("(o p) j -> p o j", p=P)
    wp_r = wp[:].rearrange("(o p) e -> p o e", p=P)
    yT_r = yT[:].rearrange("(o p) t -> p o t", p=P)

    with tile.TileContext(nc) as tc:
        with (
            tc.tile_pool(name="const", bufs=1) as constp,
            tc.tile_pool(name="xt", bufs=6) as xtp,
            tc.tile_pool(name="wqk", bufs=4) as wqkp,
            tc.tile_pool(name="wv", bufs=1) as wvp,
            tc.tile_pool(name="qkt", bufs=1) as qkTp,
            tc.tile_pool(name="vaug", bufs=1) as vap,
            tc.tile_pool(name="raw", bufs=1) as rawp,
            tc.tile_pool(name="wp", bufs=6) as wpp,
            tc.tile_pool(name="exp", bufs=3) as expp,
            tc.tile_pool(name="rr", bufs=3) as rrp,
            tc.tile_pool(name="yt", bufs=3) as ytp,
            tc.tile_pool(name="yta", bufs=6) as ytap,
            tc.tile_pool(name="psA", bufs=2, space="PSUM") as psA,
            tc.tile_pool(name="psB", bufs=2, space="PSUM") as psB,
        ):
            # ---- resident tensors (DMA issue order matters: x halves
            # first so qkT can start, then the g=0 weights, then wv) ----
            xts = []
            for cc in range(CC):
                xt_t = xtp.tile([P, T], F32R, tag="xt", name=f"xt{cc}")
                nc.sync.dma_start(xt_t[:, 0:TQ], xT_r[:, cc, 0:TQ])
                xts.append(xt_t)
            for cc in range(CC):
                nc.sync.dma_start(xts[cc][:, TQ:T], xT_r[:, cc, TQ:T])

            bqk_sb = constp.tile([P, JQK], F32)
            nc.sync.dma_start(bqk_sb[:], bqk[:])
            bp_sb = constp.tile([P, EC], F32)
            nc.sync.dma_start(bp_sb[:], bp[:])
            tri_sb = constp.tile([P, 2, P], BF16)
            nc.sync.dma_start(tri_sb[:], tri[:])
            id64_sb = constp.tile([P, HD], BF16)
            nc.sync.dma_start(id64_sb[:], id64[:])

            qkT_sb = qkTp.tile([P, JQK, T], BF16)
            v_sb = vap.tile([P, NTK, G * VW + VPAD], BF16)
            v4 = v_sb[:, :, : G * VW].rearrange("p n (g w) -> p n g w", w=VW)
            rawT = rawp.tile([P, CC, T], F32R)

            # ---- v[t, j] = x Wv, interleaved per head pair ------------
            # ones columns feed the softmax-denominator trick; tail pad is
            # junk-read by the odd head's M=128 lhsT (rows 65+ of its psum)
            onec = constp.tile([P, 1], F32)
            nc.vector.memset(onec[:], 1.0)
            zeroc = constp.tile([P, 1], F32)
            nc.vector.memset(zeroc[:], 0.0)
            ones_src = onec[:, None, None, :].to_broadcast([P, NTK, G, 1])
            nc.any.tensor_copy(v4[:, :, :, HD : HD + 1], ones_src)
            nc.any.tensor_copy(v4[:, :, :, VW - 1 : VW], ones_src)
            nc.any.tensor_copy(
                v_sb[:, :, G * VW :],
                zeroc[:, None, :].to_broadcast([P, NTK, VPAD]),
            )

            wv_sb = wvp.tile([P, CC, C], F32R)

            def v_part(tcs):
                for tc_i in tcs:
                    for jn in range(C // JV):
                        ps = psB.tile([P, TQ], F32, tag="mm", name="psv")
                        for cc in range(CC):
                            nc.tensor.matmul(
                                ps[:, :JV],
                                xts[cc][:, ts(tc_i, P)],
                                wv_sb[:, cc, ts(jn, JV)],
                                start=(cc == 0),
                                stop=(cc == CC - 1),
                            )
                        g0 = jn * (JV // P)  # 3 head pairs per 384 cols
                        srcv = ps[:, :JV].rearrange(
                            "p (g h d) -> p g h d", h=2, d=HD
                        )
                        # psum -> interleaved sbuf layout on the Act engine
                        # (DVE stays free for the divide chains); the even
                        # and odd head halves are one strided 4D AP
                        dstv = v4[:, tc_i, g0 : g0 + 3, :].rearrange(
                            "p g (h w) -> p g h w", h=2
                        )[:, :, :, :HD]
                        nc.scalar.activation(dstv, srcv, AF.Identity)

            def load_wt(jc):
                wt = wqkp.tile([P, CC, P], F32R, tag="wqk", name="wt")
                nc.sync.dma_start(wt[:], wqk_r[:, :, ts(jc, P)])
                return wt

            def qkt_chunk(jc, wt):
                for t2 in range(NTQ):
                    ps = psB.tile([P, TQ], F32, tag="mm", name="psq")
                    for cc in range(CC):
                        nc.tensor.matmul(
                            ps[:],
                            wt[:, cc, :],
                            xts[cc][:, ts(t2, TQ)],
                            start=(cc == 0),
                            stop=(cc == CC - 1),
                        )
                    nc.scalar.activation(
                        qkT_sb[:, jc, ts(t2, TQ)],
                        ps[:],
                        AF.Identity,
                        bias=bqk_sb[:, jc : jc + 1],
                    )

            def attn_block(g, t2):
                """Emit QK/exp/AV + psum evac + denominator gather for one
                block, software-pipelined one key chunk deep (QK of chunk
                i+1 is emitted before exp/AV of chunk i, so the PE never
                waits on the Act engine).  Returns (stage1, stage2)
                closures finishing the divide: stage1 (recip + broadcast
                issue) is for one block later, stage2 (multiplies) for two
                blocks later — by then every DMA they wait on has landed,
                so the strict-FIFO DVE queue never stalls."""
                jq, jk = g, G + g
                hi = 4 * (t2 + 1)  # causal: key chunks 0..hi-1
                avs = []
                for par in (0, 1):
                    av = psB.tile([P, TQ], F32, tag="av", name=f"av{par}")
                    avs.append(av)

                def qk_emit(tkc):
                    csr = tkc * P - t2 * TQ  # diag block start col
                    cs = max(0, csr)
                    pa = psA.tile([P, 2 * TQ], F32, tag="pa", name="pa")
                    for par in (0, 1):
                        qrow = HD * par
                        lo = cs if par == 0 else TQ
                        nc.tensor.matmul(
                            pa[:, lo : lo + TQ - cs],
                            qkT_sb[qrow : qrow + HD, jk, ts(tkc, P)],
                            qkT_sb[
                                qrow : qrow + HD,
                                jq,
                                t2 * TQ + cs : (t2 + 1) * TQ,
                            ],
                            start=True,
                            stop=(csr < 0),
                        )
                    if csr >= 0:
                        # add -1e30 above the diagonal with two K=64
                        # triangular matmuls (same row-tiled mode):
                        # pa[tk, lo+j] += tri[j, tk]
                        for par in (0, 1):
                            qrow = HD * par
                            lo = cs if par == 0 else TQ
                            for half in (0, 1):
                                nc.tensor.matmul(
                                    pa[:, lo + HD * half : lo + HD * (half + 1)],
                                    tri_sb[qrow : qrow + HD, half, :],
                                    id64_sb[qrow : qrow + HD, :],
                                    start=False,
                                    stop=(half == 1),
                                )
                    return pa, cs

                def ea_emit(pa, cs, tkc):
                    e = expp.tile([P, 2 * TQ], BF16, tag="exp", name="e")
                    nc.scalar.activation(
                        e[:, cs : 2 * TQ - cs],
                        pa[:, cs : 2 * TQ - cs],
                        AF.Exp,
                        scale=SCALE,
                    )
                    for par in (0, 1):
                        lo = cs if par == 0 else TQ
                        vlo = g * VW + (HD + 1) * par
                        nc.tensor.matmul(
                            avs[par][:, cs:],
                            v_sb[:, tkc, vlo : vlo + P],
                            e[:, lo : lo + TQ - cs],
                            start=(tkc == 0),
                            stop=(tkc == hi - 1),
                        )

                pend = qk_emit(0)
                for tkc in range(1, hi):
                    cur = qk_emit(tkc)
                    ea_emit(*pend, tkc - 1)
                    pend = cur
                ea_emit(*pend, hi - 1)
                # evacuate out_aug to SBUF right away in bf16 (frees the
                # psum banks fast) and gather the two denominator rows to
                # [128, 8] via DMA so the reciprocal runs on all lanes.
                asb = rrp.tile([65, 2, TQ], BF16, tag="avsb", name="asb")
                for par in (0, 1):
                    nc.vector.tensor_scalar_add(
                        asb[:, par, :], avs[par][0:65, :], 0.0
                    )
                rd = rrp.tile([P, 8], BF16, tag="rd", name="rd")
                nc.sync.dma_start(rd[:], asb[64:65, :, :])

                rrs = []

                def stage1():
                    rd2 = rrp.tile([P, 8], BF16, tag="rd2", name="rd2")
                    with nc.allow_low_precision(
                        reason="softmax denom reciprocal in bf16; rel err "
                        "~4e-3 well inside the 2e-2 budget"
                    ):
                        nc.vector.reciprocal(rd2[:], rd[:])
                    rro = rrp.tile([1, 2, TQ], BF16, tag="rro", name="rro")
                    nc.sync.dma_start(rro[0:1, :, :], rd2[:])
                    for par in (0, 1):
                        rr = rrp.tile([64, TQ], BF16, tag=f"rr{par}", name="rr")
                        nc.sync.dma_start(
                            rr[:],
                            rro[0:1, par, None, :].to_broadcast([1, 64, TQ]),
                        )
                        rrs.append(rr)

                def stage2():
                    nc.vector.tensor_tensor(
                        rawT[0:64, g, ts(t2, TQ)],
                        asb[0:64, 0, :],
                        rrs[0][:],
                        MUL,
                    )
                    tmp = rrp.tile([64, TQ], F32R, tag="otmp", name="tmp")
                    nc.vector.tensor_tensor(
                        tmp[:], asb[0:64, 1, :], rrs[1][:], MUL
                    )
                    nc.sync.dma_start(rawT[64:128, g, ts(t2, TQ)], tmp[:])

                return stage1, stage2

            def load_wpt(ec):
                wpt = wpp.tile([P, CC, P], F32R, tag="wp", name="wpt")
                nc.sync.dma_start(wpt[:], wp_r[:, :, ts(ec, P)])
                return wpt

            # The projection is split per output half into a 5-chunk
            # partial plus a single final matmul on the last head pair, so
            # no proj work ever waits on the most recent divide chain.
            def proj_partial(t2, ec, wpt):
                ps = psB.tile([P, TQ], F32, tag="mm", name="pspa")
                for jc in range(CC - 1):
                    nc.tensor.matmul(
                        ps[:],
                        wpt[:, jc, :],
                        rawT[:, jc, ts(t2, TQ)],
                        start=(jc == 0),
                        stop=(jc == CC - 2),
                    )
                yta = ytap.tile([P, TQ], F32, tag="yta", name="yta")
                nc.vector.tensor_scalar_add(yta[:], ps[:], 0.0)
                return yta

            def proj_final(t2, ec, wpt, yta):
                ps = psB.tile([P, TQ], F32, tag="mm", name="pspb")
                nc.tensor.matmul(
                    ps[:],
                    wpt[:, CC - 1, :],
                    rawT[:, CC - 1, ts(t2, TQ)],
                    start=True,
                    stop=True,
                )
                yt = ytp.tile([P, TQ], F32, tag="yt", name="yt")
                nc.vector.scalar_tensor_tensor(
                    yt[:], ps[:], bp_sb[:, ec : ec + 1], yta[:], ADD, ADD
                )
                nc.sync.dma_start(yT_r[:, ec, ts(t2, TQ)], yt[:])

            # ---- emission schedule ------------------------------------
            # startup: two qkT pairs back to back keep the PE streaming
            # while wv lands; v then fills tkc 0-3 ahead of attention.
            # `fins` holds the pending divide stages: after emitting block
            # k, run stage1 of block k-1 and stage2 of block k-2.
            fins = []

            def push_fins(s=None):
                if len(fins) >= 1 and fins[-1][0]:
                    fins[-1][0]()
                    fins[-1][0] = None
                if len(fins) >= 2 and fins[-2][1]:
                    fins[-2][1]()
                    fins[-2][1] = None
                if s is not None:
                    fins.append(list(s))

            wts = (load_wt(0), load_wt(G))
            nxt = (load_wt(1), load_wt(G + 1))
            nc.sync.dma_start(wv_sb[:], wv_r)
            qkt_chunk(0, wts[0])
            qkt_chunk(G, wts[1])
            qkt_chunk(1, nxt[0])
            qkt_chunk(G + 1, nxt[1])
            v_part(range(0, 4))
            nxt = (load_wt(2), load_wt(G + 2))
            fins.append(list(attn_block(0, 0)))
            v_part(range(4, 6))
            wpts = []
            for g in range(1, G):
                if g >= 2:
                    wts = nxt
                    qkt_chunk(g, wts[0])
                    qkt_chunk(G + g, wts[1])
                    if g < G - 1:
                        nxt = (load_wt(g + 1), load_wt(G + g + 1))
                    else:
                        wpts = [load_wpt(ec) for ec in range(EC)]
                if g == 1:
                    v_part(range(6, 8))
                s = attn_block(g, 0)
                push_fins(s)
            ytas0 = [None] * EC
            for g in range(G):
                s = attn_block(g, 1)
                push_fins(s)
                if g in (1, 2):
                    for ec in (3 * g - 3, 3 * g - 2, 3 * g - 1):
                        ytas0[ec] = proj_partial(0, ec, wpts[ec])
                elif g >= 3:
                    for ec in (2 * g - 6, 2 * g - 5):
                        proj_final(0, ec, wpts[ec], ytas0[ec])
            # drain the remaining divide stages, overlapping the t2=1
            # projection partials with the last chains
            fins[-1][0]()
            fins[-2][1]()
            ytas1 = [proj_partial(1, ec, wpts[ec]) for ec in range(EC - 1)]
            fins[-1][1]()
            ytas1.append(proj_partial(1, EC - 1, wpts[EC - 1]))
            for ec in range(EC):
                proj_final(1, ec, wpts[ec], ytas1[ec])

    nc.compile()
    _CACHE["nc"] = nc
    return nc


def _round_fp32r(a):
    """Round fp32 to fp32r (11-bit mantissa) the way the PE expects."""
    u = np.ascontiguousarray(a, dtype=np.float32).view(np.uint32)
    u = ((u.astype(np.uint64) + 0x800) & 0xFFFFF000).astype(np.uint32)
    return u.view(np.float32)


def make_in_maps(x, w_attn, b_attn, w_proj, b_proj):
    x = np.ascontiguousarray(np.asarray(x, dtype=np.float32))
    w_attn = np.ascontiguousarray(np.asarray(w_attn, dtype=np.float32))
    b_attn = np.ascontiguousarray(np.asarray(b_attn, dtype=np.float32))
    w_proj = np.ascontiguousarray(np.asarray(w_proj, dtype=np.float32))
    b_proj = np.ascontiguousarray(np.asarray(b_proj, dtype=np.float32))

    bf = ml_dtypes.bfloat16
    wqk = _round_fp32r(w_attn[:, : 2 * C])
    wv = _round_fp32r(w_attn[:, 2 * C :])
    w_proj_r = _round_fp32r(w_proj)
    bqk = np.ascontiguousarray(b_attn[: 2 * C].reshape(JQK, P).T)
    # the v bias folds into the projection bias: y = Wp.T (raw + bv) + bp
    bv = b_attn[2 * C :].astype(np.float64)
    bp_eff = (w_proj.astype(np.float64).T @ bv + b_proj).astype(np.float32)
    bp = np.ascontiguousarray(bp_eff.reshape(EC, P).T)
    # tri[qrow+r, 0, tk] masks j=r    < tk; tri[qrow+r, 1, tk] masks 64+r < tk
    tri = np.zeros((P, 2, P), dtype=bf)
    tk = np.arange(P)[None, :]
    for qrow in (0, HD):
        r = np.arange(HD)[:, None]
        tri[qrow : qrow + HD, 0, :] = np.where(r < tk, -1e30, 0.0).astype(bf)
        tri[qrow : qrow + HD, 1, :] = np.where(r + HD < tk, -1e30, 0.0).astype(
            bf
        )
    id64 = np.zeros((P, HD), dtype=bf)
    for qrow in (0, HD):
        id64[qrow : qrow + HD, :] = np.eye(HD, dtype=bf)

    shared = {
        "wqk": wqk,
        "wv": wv,
        "wp": w_proj_r,
        "bqk": bqk,
        "bp": bp,
        "tri": tri,
        "id64": id64,
    }
    return [
        {"xT": _round_fp32r(x[b].T), **shared} for b in range(NCORES)
    ]


def kernel(**inputs):
    nc = _build()
    in_maps = make_in_maps(
        inputs["x"],
        inputs["w_attn"],
        inputs["b_attn"],
        inputs["w_proj"],
        inputs["b_proj"],
    )
    res = run_bass_kernel_spmd(nc, in_maps, list(range(NCORES)))
    out = np.stack(
        [np.ascontiguousarray(res.results[b]["yT"].T) for b in range(NCORES)]
    )
    return out.astype(np.float32)


# revision 14
# speedup vs baseline: 1.0025x; 1.0016x over previous
"""Causal self-attention (B=8, T=1024, C=768, NH=12) on 8 TRN2 NeuronCores.

Strategy: pure batch data-parallel — core b computes batch element b end to
end (no collectives).

PE cost model (instruction_cost_v2 + HW traces): matmul streaming cost =
N output cols x 0.417ns, serial per matmul (no tile overlap observed on
HW), plus ~107ns LDWEIGHTS hidden only behind a long previous stream.
So the kernel minimizes matmul COUNT and total output columns:
  - no PE mask matmuls: the causal diagonal block is zeroed post-exp on
    the otherwise-idle GpSimd engine (e *= upper-tri 0/1 mask)
  - everything bf16 (same 1 cyc/row as fp32r, half the DMA/SBUF)

Engines are strict-FIFO, so a long-latency wait at the head of a queue
blocks everything behind it.  The softmax divide chain is therefore
split across blocks: block k emits evac + denominator-gather DMA,
stage1 (recip + broadcast issue) runs after block k+1, stage2 (the
multiplies) after block k+2 — every DMA waited on has already landed.

Per-core dataflow (everything kept "transposed", i.e. [feature, time]):
  xT [C, T] bf16                             (host pre-transposes x[b])
  qkT[j, t] = Wqk[:, j].T x  (bf16)          psum evac on Act (+bias)
  v  [t, j] = x Wv           (bf16)          AV-friendly layout, augmented
                                             with a ones column per head
  attT[tk, tq] = kT.T @ qT   per head pair   K=64 matmuls; the two
                                             parities land at psum cols
                                             [cs,TQ) and [TQ,2TQ-cs)
  expT = exp(scale * attT)   one Act op over the contiguous [cs, 2TQ-cs)
  diag upper-tri of expT zeroed on GpSimd
  out_aug = [v | 1 | junk].T @ expT          row 64 = softmax denominator
  rawT[j, t] = out_aug[d] * (1/denom)        denom broadcast via DMA (bf16)
  yT[e, t] = Wp.T @ rawT + bp'               bp' = Wp.T bv + bp (host);
                                             split 5-chunk partial + final
                                             matmul so the tail never waits
                                             on the last divide chain.
"""

import os
import sys

import numpy as np

for _p in ("/opt/trn_rl_repo", "/root/.axon_site/_ro/trn_rl_repo"):
    if os.path.isdir(_p) and _p not in sys.path:
        sys.path.insert(0, _p)

import ml_dtypes

import concourse.bacc as bacc
import concourse.mybir as mybir
import concourse.tile as tile
from concourse.bass import ts
from concourse.bass_utils import run_bass_kernel_spmd

B, T, C = 8, 1024, 768
NH, HD = 12, 64
P = 128
NCORES = 8
CC = C // P            # 6 contraction chunks over C
JQK = 2 * C // P       # 12 output chunks for q|k
EC = C // P            # 6 output chunks for the projection
TQ = 512               # moving-dim tile (max psum bank width)
NTQ = T // TQ          # 2
NTK = T // P           # 8 key chunks
G = NH // 2            # 6 head pairs (two 64-wide heads per 128 partitions)
VW = 2 * HD + 2        # 130: per-pair v layout [d_even(64), 1, d_odd(64), 1]
VPAD = 63              # tail pad so the odd lhsT can always grab 128 cols
JV = 384               # v output tile width (3 head pairs)
SCALE = 1.0 / float(np.sqrt(HD))
F32 = mybir.dt.float32
BF16 = mybir.dt.bfloat16
AF = mybir.ActivationFunctionType
ADD = mybir.AluOpType.add
MUL = mybir.AluOpType.mult

_CACHE = {}


def _build():
    if "nc" in _CACHE:
        return _CACHE["nc"]

    nc = bacc.Bacc("TRN2", target_bir_lowering=False, debug=False)

    xT = nc.dram_tensor("xT", [C, T], BF16, kind="ExternalInput")
    wqk = nc.dram_tensor("wqk", [C, 2 * C], BF16, kind="ExternalInput")
    wv = nc.dram_tensor("wv", [C, C], BF16, kind="ExternalInput")
    wp = nc.dram_tensor("wp", [C, C], BF16, kind="ExternalInput")
    bqk = nc.dram_tensor("bqk", [P, JQK], F32, kind="ExternalInput")
    bp = nc.dram_tensor("bp", [P, EC], F32, kind="ExternalInput")
    m01 = nc.dram_tensor("m01", [P, P], BF16, kind="ExternalInput")
    yT = nc.dram_tensor("yT", [C, T], F32, kind="ExternalOutput")

    xT_r = xT[:].rearrange("(o p) t -> p o t", p=P)
    wqk_r = wqk[:].rearrange("(o p) j -> p o j", p=P)
    wv_r = wv[:].rearrange("(o p) j -> p o j", p=P)
    wp_r = wp[:].rearrange("(o p) e -> p o e", p=P)
    yT_r = yT[:].rearrange("(o p) t -> p o t", p=P)

    with tile.TileContext(nc) as tc:
        with (
            tc.tile_pool(name="const", bufs=1) as constp,
            tc.tile_pool(name="xt", bufs=6) as xtp,
            tc.tile_pool(name="wqk", bufs=4) as wqkp,
            tc.tile_pool(name="wv", bufs=1) as wvp,
            tc.tile_pool(name="qkt", bufs=1) as qkTp,
            tc.tile_pool(name="vaug", bufs=1) as vap,
            tc.tile_pool(name="raw", bufs=1) as rawp,
            tc.tile_pool(name="wp", bufs=6) as wpp,
            tc.tile_pool(name="exp", bufs=3) as expp,
            tc.tile_pool(name="rr", bufs=3) as rrp,
            tc.tile_pool(name="yt", bufs=3) as ytp,
            tc.tile_pool(name="yta", bufs=12) as ytap,
            tc.tile_pool(name="psA", bufs=2, space="PSUM") as psA,
            tc.tile_pool(name="psB", bufs=2, space="PSUM") as psB,
        ):
            # ---- resident tensors (DMA issue order matters: x halves and
            # the g=0 weights first so the PE can start, then wv) --------
            xts = []
            for cc in range(CC):
                xt_t = xtp.tile([P, T], BF16, tag="xt", name=f"xt{cc}")
                nc.sync.dma_start(xt_t[:, 0:TQ], xT_r[:, cc, 0:TQ])
                xts.append(xt_t)
            for cc in range(CC):
                nc.sync.dma_start(xts[cc][:, TQ:T], xT_r[:, cc, TQ:T])

            bqk_sb = constp.tile([P, JQK], F32)
            nc.sync.dma_start(bqk_sb[:], bqk[:])
            bp_sb = constp.tile([P, EC], F32)
            nc.sync.dma_start(bp_sb[:], bp[:])
            m01_sb = constp.tile([P, P], BF16)
            nc.sync.dma_start(m01_sb[:], m01[:])

            qkT_sb = qkTp.tile([P, JQK, T], BF16)
            v_sb = vap.tile([P, NTK, G * VW + VPAD], BF16)
            v4 = v_sb[:, :, : G * VW].rearrange("p n (g w) -> p n g w", w=VW)
            rawT = rawp.tile([P, CC, T], BF16)

            # ---- v[t, j] = x Wv, interleaved per head pair ------------
            # ones columns feed the softmax-denominator trick; tail pad is
            # junk-read by the odd head's M=128 lhsT (rows 65+ of its psum)
            onec = constp.tile([P, 1], F32)
            nc.vector.memset(onec[:], 1.0)
            zeroc = constp.tile([P, 1], F32)
            nc.vector.memset(zeroc[:], 0.0)
            ones_src = onec[:, None, None, :].to_broadcast([P, NTK, G, 1])
            nc.any.tensor_copy(v4[:, :, :, HD : HD + 1], ones_src)
            nc.any.tensor_copy(v4[:, :, :, VW - 1 : VW], ones_src)
            nc.any.tensor_copy(
                v_sb[:, :, G * VW :],
                zeroc[:, None, :].to_broadcast([P, NTK, VPAD]),
            )

            wv_sb = wvp.tile([P, CC, C], BF16)

            def v_part(tcs):
                for tc_i in tcs:
                    for jn in range(C // JV):
                        ps = psB.tile([P, TQ], F32, tag="mm", name="psv")
                        for cc in range(CC):
                            nc.tensor.matmul(
                                ps[:, :JV],
                                xts[cc][:, ts(tc_i, P)],
                                wv_sb[:, cc, ts(jn, JV)],
                                start=(cc == 0),
                                stop=(cc == CC - 1),
                            )
                        g0 = jn * (JV // P)  # 3 head pairs per 384 cols
                        srcv = ps[:, :JV].rearrange(
                            "p (g h d) -> p g h d", h=2, d=HD
                        )
                        # psum -> interleaved sbuf layout on the Act engine
                        # (DVE stays free for the divide chains); the even
                        # and odd head halves are one strided 4D AP
                        dstv = v4[:, tc_i, g0 : g0 + 3, :].rearrange(
                            "p g (h w) -> p g h w", h=2
                        )[:, :, :, :HD]
                        nc.scalar.activation(dstv, srcv, AF.Identity)

            def load_wt(jc):
                wt = wqkp.tile([P, CC, P], BF16, tag="wqk", name="wt")
                nc.sync.dma_start(wt[:], wqk_r[:, :, ts(jc, P)])
                return wt

            def qkt_chunk(jc, wt):
                for t2 in range(NTQ):
                    ps = psB.tile([P, TQ], F32, tag="mm", name="psq")
                    for cc in range(CC):
                        nc.tensor.matmul(
                            ps[:],
                            wt[:, cc, :],
                            xts[cc][:, ts(t2, TQ)],
                            start=(cc == 0),
                            stop=(cc == CC - 1),
                        )
                    nc.scalar.activation(
                        qkT_sb[:, jc, ts(t2, TQ)],
                        ps[:],
                        AF.Identity,
                        bias=bqk_sb[:, jc : jc + 1],
                    )

            def attn_block(g, t2):
                """Emit QK/exp/mask/AV + psum evac + denominator gather,
                software-pipelined one key chunk deep (QK of chunk i+1 is
                emitted before exp/AV of chunk i so the PE never waits on
                the Act engine).  Returns (stage1, stage2) divide-chain
                closures to be emitted one and two blocks later."""
                jq, jk = g, G + g
                hi = 4 * (t2 + 1)  # causal: key chunks 0..hi-1
                avs = []
                for par in (0, 1):
                    av = psB.tile([P, TQ], F32, tag="av", name=f"av{par}")
                    avs.append(av)

                def qk_emit(tkc):
                    csr = tkc * P - t2 * TQ  # diag block start col
                    cs = max(0, csr)
                    pa = psA.tile([P, 2 * TQ], F32, tag="pa", name="pa")
                    for par in (0, 1):
                        qrow = HD * par
                        lo = cs if par == 0 else TQ
                        nc.tensor.matmul(
                            pa[:, lo : lo + TQ - cs],
                            qkT_sb[qrow : qrow + HD, jk, ts(tkc, P)],
                            qkT_sb[
                                qrow : qrow + HD,
                                jq,
                                t2 * TQ + cs : (t2 + 1) * TQ,
                            ],
                            start=True,
                            stop=True,
                        )
                    return pa, cs, csr >= 0

                def ea_emit(pa, cs, diag, tkc):
                    e = expp.tile([P, 2 * TQ], BF16, tag="exp", name="e")
                    nc.scalar.activation(
                        e[:, cs : 2 * TQ - cs],
                        pa[:, cs : 2 * TQ - cs],
                        AF.Exp,
                        scale=SCALE,
                    )
                    if diag:
                        # zero the strictly-upper triangle of the diagonal
                        # 128-block (e[tk, j] for j < tk) on GpSimd — off
                        # both the PE and the busy Act/DVE engines
                        for lo in (cs, TQ):
                            nc.gpsimd.tensor_tensor(
                                e[:, lo : lo + P],
                                e[:, lo : lo + P],
                                m01_sb[:],
                                MUL,
                            )
                    for par in (0, 1):
                        lo = cs if par == 0 else TQ
                        vlo = g * VW + (HD + 1) * par
                        nc.tensor.matmul(
                            avs[par][:, cs:],
                            v_sb[:, tkc, vlo : vlo + P],
                            e[:, lo : lo + TQ - cs],
                            start=(tkc == 0),
                            stop=(tkc == hi - 1),
                        )

                pend = qk_emit(0)
                for tkc in range(1, hi):
                    cur = qk_emit(tkc)
                    ea_emit(*pend, tkc - 1)
                    pend = cur
                ea_emit(*pend, hi - 1)
                # evacuate out_aug to SBUF right away in bf16 (frees the
                # psum banks fast) and gather the two denominator rows to
                # [128, 8] via DMA so the reciprocal runs on all lanes.
                asb = rrp.tile([65, 2, TQ], BF16, tag="avsb", name="asb")
                for par in (0, 1):
                    nc.vector.tensor_scalar_add(
                        asb[:, par, :], avs[par][0:65, :], 0.0
                    )
                rd = rrp.tile([P, 8], BF16, tag="rd", name="rd")
                nc.sync.dma_start(rd[:], asb[64:65, :, :])

                rrs = []

                def stage1():
                    rd2 = rrp.tile([P, 8], BF16, tag="rd2", name="rd2")
                    with nc.allow_low_precision(
                        reason="softmax denom reciprocal in bf16; rel err "
                        "~4e-3 well inside the 2e-2 budget"
                    ):
                        nc.vector.reciprocal(rd2[:], rd[:])
                    rro = rrp.tile([1, 2, TQ], BF16, tag="rro", name="rro")
                    nc.sync.dma_start(rro[0:1, :, :], rd2[:])
                    for par in (0, 1):
                        rr = rrp.tile([64, TQ], BF16, tag=f"rr{par}", name="rr")
                        nc.sync.dma_start(
                            rr[:],
                            rro[0:1, par, None, :].to_broadcast([1, 64, TQ]),
                        )
                        rrs.append(rr)

                def stage2():
                    nc.vector.tensor_tensor(
                        rawT[0:64, g, ts(t2, TQ)],
                        asb[0:64, 0, :],
                        rrs[0][:],
                        MUL,
                    )
                    tmp = rrp.tile([64, TQ], BF16, tag="otmp", name="tmp")
                    nc.vector.tensor_tensor(
                        tmp[:], asb[0:64, 1, :], rrs[1][:], MUL
                    )
                    nc.sync.dma_start(rawT[64:128, g, ts(t2, TQ)], tmp[:])

                return stage1, stage2

            def load_wpt(ec):
                wpt = wpp.tile([P, CC, P], BF16, tag="wp", name="wpt")
                nc.sync.dma_start(wpt[:], wp_r[:, :, ts(ec, P)])
                return wpt

            # The projection is split per output half into a 5-chunk
            # partial plus a single final matmul on the last head pair, so
            # no proj work ever waits on the most recent divide chain.
            def proj_partial(t2, ec, wpt):
                ps = psB.tile([P, TQ], F32, tag="mm", name="pspa")
                for jc in range(CC - 1):
                    nc.tensor.matmul(
                        ps[:],
                        wpt[:, jc, :],
                        rawT[:, jc, ts(t2, TQ)],
                        start=(jc == 0),
                        stop=(jc == CC - 2),
                    )
                yta = ytap.tile([P, TQ], F32, tag="yta", name="yta")
                nc.vector.tensor_scalar_add(yta[:], ps[:], 0.0)
                return yta

            def proj_final(t2, ec, wpt, yta):
                ps = psB.tile([P, TQ], F32, tag="mm", name="pspb")
                nc.tensor.matmul(
                    ps[:],
                    wpt[:, CC - 1, :],
                    rawT[:, CC - 1, ts(t2, TQ)],
                    start=True,
                    stop=True,
                )
                yt = ytp.tile([P, TQ], F32, tag="yt", name="yt")
                nc.vector.scalar_tensor_tensor(
                    yt[:], ps[:], bp_sb[:, ec : ec + 1], yta[:], ADD, ADD
                )
                nc.sync.dma_start(yT_r[:, ec, ts(t2, TQ)], yt[:])

            # ---- emission schedule ------------------------------------
            # one merged loop: per head pair g, the PE-heavy qkT units and
            # both attention halves interleave, so the Act-bound exp work
            # always has PE work to hide behind.  `fins` holds the pending
            # divide stages: after emitting block k, run stage1 of block
            # k-1 and stage2 of block k-2.
            fins = []

            def push_fins(s=None):
                if len(fins) >= 1 and fins[-1][0]:
                    fins[-1][0]()
                    fins[-1][0] = None
                if len(fins) >= 2 and fins[-2][1]:
                    fins[-2][1]()
                    fins[-2][1] = None
                if s is not None:
                    fins.append(list(s))

            wts = (load_wt(0), load_wt(G))
            nc.sync.dma_start(wv_sb[:], wv_r)
            nxt = (load_wt(1), load_wt(G + 1))
            qkt_chunk(0, wts[0])
            qkt_chunk(G, wts[1])
            v_part(range(0, 4))
            qkt_chunk(1, nxt[0])
            qkt_chunk(G + 1, nxt[1])
            v_part(range(4, 8))
            nxt = (load_wt(2), load_wt(G + 2))
            ytas0 = [None] * EC
            ytas1 = [None] * EC
            wpts = []
            for g in range(G):
                if g >= 2:
                    wts = nxt
                    qkt_chunk(g, wts[0])
                    qkt_chunk(G + g, wts[1])
                    if g < G - 1:
                        nxt = (load_wt(g + 1), load_wt(G + g + 1))
                    else:
                        wpts = [load_wpt(ec) for ec in range(EC)]
                push_fins(attn_block(g, 0))
                push_fins(attn_block(g, 1))
                if g == G - 1:
                    # rawT[:, 0:5, t2=0] is complete (stage2 of block (4,0)
                    # was emitted just before attn(5,0)): overlap the t2=0
                    # partial projections with the last attention block
                    for ec in range(EC):
                        ytas0[ec] = proj_partial(0, ec, wpts[ec])
            # drain: stage1(5,1) then stage2(5,0) -> t2=0 finals; the t2=1
            # partials wait only on stage2(4,1); finals on stage2(5,1).
            fins[-1][0]()
            fins[-2][1]()
            for ec in range(EC):
                proj_final(0, ec, wpts[ec], ytas0[ec])
            for ec in range(EC - 1):
                ytas1[ec] = proj_partial(1, ec, wpts[ec])
            fins[-1][1]()
            ytas1[EC - 1] = proj_partial(1, EC - 1, wpts[EC - 1])
            for ec in range(EC):
                proj_final(1, ec, wpts[ec], ytas1[ec])

    nc.compile()
    _CACHE["nc"] = nc
    return nc


def make_in_maps(x, w_attn, b_attn, w_proj, b_proj):
    x = np.ascontiguousarray(np.asarray(x, dtype=np.float32))
    w_attn = np.ascontiguousarray(np.asarray(w_attn, dtype=np.float32))
    b_attn = np.ascontiguousarray(np.asarray(b_attn, dtype=np.float32))
    w_proj = np.ascontiguousarray(np.asarray(w_proj, dtype=np.float32))
    b_proj = np.ascontiguousarray(np.asarray(b_proj, dtype=np.float32))

    bf = ml_dtypes.bfloat16
    wqk = w_attn[:, : 2 * C].astype(bf)
    wv = w_attn[:, 2 * C :].astype(bf)
    wp_b = w_proj.astype(bf)
    bqk = np.ascontiguousarray(b_attn[: 2 * C].reshape(JQK, P).T)
    # the v bias folds into the projection bias: y = Wp.T (raw + bv) + bp
    bv = b_attn[2 * C :].astype(np.float64)
    bp_eff = (w_proj.astype(np.float64).T @ bv + b_proj).astype(np.float32)
    bp = np.ascontiguousarray(bp_eff.reshape(EC, P).T)
    # m01[p, j] = 1 where j >= p (keep), 0 above-diagonal columns
    m01 = (np.arange(P)[None, :] >= np.arange(P)[:, None]).astype(bf)

    shared = {
        "wqk": wqk,
        "wv": wv,
        "wp": wp_b,
        "bqk": bqk,
        "bp": bp,
        "m01": m01,
    }
    return [
        {"xT": np.ascontiguousarray(x[b].T).astype(bf), **shared}
        for b in range(NCORES)
    ]


def kernel(**inputs):
    nc = _build()
    in_maps = make_in_maps(
        inputs["x"],
        inputs["w_attn"],
        inputs["b_attn"],
        inputs["w_proj"],
        inputs["b_proj"],
    )
    res = run_bass_kernel_spmd(nc, in_maps, list(range(NCORES)))
    out = np.stack(
        [np.ascontiguousarray(res.results[b]["yT"].T) for b in range(NCORES)]
    )
    return out.astype(np.float32)


# revision 18
# speedup vs baseline: 1.0265x; 1.0239x over previous
"""Causal self-attention (B=8, T=1024, C=768, NH=12) on 8 TRN2 NeuronCores.

Strategy: pure batch data-parallel — core b computes batch element b end to
end (no collectives).

PE cost model (instruction_cost_v2 + HW traces): matmul streaming cost =
N output cols x 0.417ns, serial per matmul (no tile overlap observed on
HW), plus ~107ns LDWEIGHTS hidden only behind a long previous stream.
So the kernel minimizes matmul COUNT and total output columns:
  - no PE mask matmuls: the causal diagonal block is zeroed post-exp on
    the otherwise-idle GpSimd engine (e *= upper-tri 0/1 mask)
  - everything bf16 (same 1 cyc/row as fp32r, half the DMA/SBUF)

Engines are strict-FIFO, so a long-latency wait at the head of a queue
blocks everything behind it.  The softmax divide chain is therefore
split across blocks: block k emits evac + denominator-gather DMA,
stage1 (recip + broadcast issue) runs after block k+1, stage2 (the
multiplies) after block k+2 — every DMA waited on has already landed.

Per-core dataflow (everything kept "transposed", i.e. [feature, time]):
  xT [C, T] bf16                             (host pre-transposes x[b])
  qkT[j, t] = Wqk[:, j].T x  (bf16)          psum evac on Act (+bias)
  v  [t, j] = x Wv           (bf16)          AV-friendly layout, augmented
                                             with a ones column per head
  attT[tk, tq] = kT.T @ qT   per head pair   K=64 matmuls; the two
                                             parities land at psum cols
                                             [cs,TQ) and [TQ,2TQ-cs)
  expT = exp(scale * attT)   one Act op over the contiguous [cs, 2TQ-cs)
  diag upper-tri of expT zeroed on GpSimd
  out_aug = [v | 1 | junk].T @ expT          row 64 = softmax denominator
  rawT[j, t] = out_aug[d] * (1/denom)        denom broadcast via DMA (bf16)
  yT[e, t] = Wp.T @ rawT + bp'               bp' = Wp.T bv + bp (host);
                                             split 5-chunk partial + final
                                             matmul so the tail never waits
                                             on the last divide chain.
"""

import os
import sys
from collections import deque

import numpy as np

for _p in ("/opt/trn_rl_repo", "/root/.axon_site/_ro/trn_rl_repo"):
    if os.path.isdir(_p) and _p not in sys.path:
        sys.path.insert(0, _p)

import ml_dtypes

import concourse.bacc as bacc
import concourse.mybir as mybir
import concourse.tile as tile
from concourse.bass import ts
from concourse.bass_utils import run_bass_kernel_spmd

B, T, C = 8, 1024, 768
NH, HD = 12, 64
P = 128
NCORES = 8
CC = C // P            # 6 contraction chunks over C
JQK = 2 * C // P       # 12 output chunks for q|k
EC = C // P            # 6 output chunks for the projection
TQ = 512               # moving-dim tile (max psum bank width)
NTQ = T // TQ          # 2
NTK = T // P           # 8 key chunks
G = NH // 2            # 6 head pairs (two 64-wide heads per 128 partitions)
VW = 2 * HD + 2        # 130: per-pair v layout [d_even(64), 1, d_odd(64), 1]
VPAD = 63              # tail pad so the odd lhsT can always grab 128 cols
JV = 384               # v output tile width (3 head pairs)
SCALE = 1.0 / float(np.sqrt(HD))
F32 = mybir.dt.float32
BF16 = mybir.dt.bfloat16
AF = mybir.ActivationFunctionType
ADD = mybir.AluOpType.add
MUL = mybir.AluOpType.mult

_CACHE = {}


def _build():
    if "nc" in _CACHE:
        return _CACHE["nc"]

    nc = bacc.Bacc("TRN2", target_bir_lowering=False, debug=False)

    xT = nc.dram_tensor("xT", [C, T], BF16, kind="ExternalInput")
    wqk = nc.dram_tensor("wqk", [C, 2 * C], BF16, kind="ExternalInput")
    wv = nc.dram_tensor("wv", [C, C], BF16, kind="ExternalInput")
    wp = nc.dram_tensor("wp", [C, C], BF16, kind="ExternalInput")
    bqk = nc.dram_tensor("bqk", [P, JQK], F32, kind="ExternalInput")
    bp = nc.dram_tensor("bp", [P, EC], F32, kind="ExternalInput")
    m01 = nc.dram_tensor("m01", [P, P], BF16, kind="ExternalInput")
    yT = nc.dram_tensor("yT", [C, T], F32, kind="ExternalOutput")

    xT_r = xT[:].rearrange("(o p) t -> p o t", p=P)
    wqk_r = wqk[:].rearrange("(o p) j -> p o j", p=P)
    wv_r = wv[:].rearrange("(o p) j -> p o j", p=P)
    wp_r = wp[:].rearrange("(o p) e -> p o e", p=P)
    yT_r = yT[:].rearrange("(o p) t -> p o t", p=P)

    with tile.TileContext(nc) as tc:
        with (
            tc.tile_pool(name="const", bufs=1) as constp,
            tc.tile_pool(name="xt", bufs=6) as xtp,
            tc.tile_pool(name="wqk", bufs=4) as wqkp,
            tc.tile_pool(name="wv", bufs=1) as wvp,
            tc.tile_pool(name="qkt", bufs=1) as qkTp,
            tc.tile_pool(name="vaug", bufs=1) as vap,
            tc.tile_pool(name="raw", bufs=1) as rawp,
            tc.tile_pool(name="wp", bufs=6) as wpp,
            tc.tile_pool(name="exp", bufs=3) as expp,
            tc.tile_pool(name="rr", bufs=3) as rrp,
            tc.tile_pool(name="yt", bufs=3) as ytp,
            tc.tile_pool(name="yta", bufs=12) as ytap,
            tc.tile_pool(name="psA", bufs=2, space="PSUM") as psA,
            tc.tile_pool(name="psB", bufs=2, space="PSUM") as psB,
        ):
            # ---- resident tensors (DMA issue order matters: x halves and
            # the g=0 weights first so the PE can start, then wv) --------
            xts = []
            for cc in range(CC):
                xt_t = xtp.tile([P, T], BF16, tag="xt", name=f"xt{cc}")
                nc.sync.dma_start(xt_t[:, 0:TQ], xT_r[:, cc, 0:TQ])
                xts.append(xt_t)
            for cc in range(CC):
                nc.sync.dma_start(xts[cc][:, TQ:T], xT_r[:, cc, TQ:T])

            bqk_sb = constp.tile([P, JQK], F32)
            nc.sync.dma_start(bqk_sb[:], bqk[:])
            bp_sb = constp.tile([P, EC], F32)
            nc.sync.dma_start(bp_sb[:], bp[:])
            m01_sb = constp.tile([P, P], BF16)
            nc.sync.dma_start(m01_sb[:], m01[:])

            qkT_sb = qkTp.tile([P, JQK, T], BF16)
            v_sb = vap.tile([P, NTK, G * VW + VPAD], BF16)
            v4 = v_sb[:, :, : G * VW].rearrange("p n (g w) -> p n g w", w=VW)
            rawT = rawp.tile([P, CC, T], BF16)

            # ---- v[t, j] = x Wv, interleaved per head pair ------------
            # ones columns feed the softmax-denominator trick; tail pad is
            # junk-read by the odd head's M=128 lhsT (rows 65+ of its psum)
            onec = constp.tile([P, 1], F32)
            nc.vector.memset(onec[:], 1.0)
            zeroc = constp.tile([P, 1], F32)
            nc.vector.memset(zeroc[:], 0.0)
            ones_src = onec[:, None, None, :].to_broadcast([P, NTK, G, 1])
            nc.any.tensor_copy(v4[:, :, :, HD : HD + 1], ones_src)
            nc.any.tensor_copy(v4[:, :, :, VW - 1 : VW], ones_src)
            nc.any.tensor_copy(
                v_sb[:, :, G * VW :],
                zeroc[:, None, :].to_broadcast([P, NTK, VPAD]),
            )

            wv_sb = wvp.tile([P, CC, C], BF16)

            def v_part(tcs):
                for tc_i in tcs:
                    for jn in range(C // JV):
                        ps = psB.tile([P, TQ], F32, tag="mm", name="psv")
                        for cc in range(CC):
                            nc.tensor.matmul(
                                ps[:, :JV],
                                xts[cc][:, ts(tc_i, P)],
                                wv_sb[:, cc, ts(jn, JV)],
                                start=(cc == 0),
                                stop=(cc == CC - 1),
                            )
                        g0 = jn * (JV // P)  # 3 head pairs per 384 cols
                        srcv = ps[:, :JV].rearrange(
                            "p (g h d) -> p g h d", h=2, d=HD
                        )
                        # psum -> interleaved sbuf layout on the Act engine
                        # (DVE stays free for the divide chains); the even
                        # and odd head halves are one strided 4D AP
                        dstv = v4[:, tc_i, g0 : g0 + 3, :].rearrange(
                            "p g (h w) -> p g h w", h=2
                        )[:, :, :, :HD]
                        nc.scalar.activation(dstv, srcv, AF.Identity)

            def load_wt(jc):
                wt = wqkp.tile([P, CC, P], BF16, tag="wqk", name="wt")
                nc.sync.dma_start(wt[:], wqk_r[:, :, ts(jc, P)])
                return wt

            def qkt_chunk(jc, wt):
                for t2 in range(NTQ):
                    ps = psB.tile([P, TQ], F32, tag="mm", name="psq")
                    for cc in range(CC):
                        nc.tensor.matmul(
                            ps[:],
                            wt[:, cc, :],
                            xts[cc][:, ts(t2, TQ)],
                            start=(cc == 0),
                            stop=(cc == CC - 1),
                        )
                    nc.scalar.activation(
                        qkT_sb[:, jc, ts(t2, TQ)],
                        ps[:],
                        AF.Identity,
                        bias=bqk_sb[:, jc : jc + 1],
                    )

            # ---- filler drip queue ------------------------------------
            # The PE executes its queue strictly in order, so the only way
            # to fill its exp-wait bubbles inside an attention block is to
            # interleave independent single matmuls BETWEEN the block's
            # matmuls.  fillq holds one-matmul closures; attention blocks
            # pop a few at each key chunk.
            fillq = deque()

            def drip(n=1):
                for _ in range(n):
                    if not fillq:
                        break
                    fillq.popleft()()

            def flushq():
                while fillq:
                    fillq.popleft()()

            def attn_block(g, t2):
                """Emit QK/exp/mask/AV + psum evac + denominator gather,
                software-pipelined one key chunk deep (QK of chunk i+1 is
                emitted before exp/AV of chunk i so the PE never waits on
                the Act engine).  Returns (stage1, stage2) divide-chain
                closures to be emitted one and two blocks later."""
                jq, jk = g, G + g
                hi = 4 * (t2 + 1)  # causal: key chunks 0..hi-1
                avs = []
                for par in (0, 1):
                    av = psB.tile([P, TQ], F32, tag="av", name=f"av{par}")
                    avs.append(av)

                def qk_emit(tkc):
                    csr = tkc * P - t2 * TQ  # diag block start col
                    cs = max(0, csr)
                    pa = psA.tile([P, 2 * TQ], F32, tag="pa", name="pa")
                    for par in (0, 1):
                        qrow = HD * par
                        lo = cs if par == 0 else TQ
                        nc.tensor.matmul(
                            pa[:, lo : lo + TQ - cs],
                            qkT_sb[qrow : qrow + HD, jk, ts(tkc, P)],
                            qkT_sb[
                                qrow : qrow + HD,
                                jq,
                                t2 * TQ + cs : (t2 + 1) * TQ,
                            ],
                            start=True,
                            stop=True,
                        )
                    return pa, cs, csr >= 0

                def ea_emit(pa, cs, diag, tkc):
                    e = expp.tile([P, 2 * TQ], BF16, tag="exp", name="e")
                    nc.scalar.activation(
                        e[:, cs : 2 * TQ - cs],
                        pa[:, cs : 2 * TQ - cs],
                        AF.Exp,
                        scale=SCALE,
                    )
                    if diag:
                        # zero the strictly-upper triangle of the diagonal
                        # 128-block (e[tk, j] for j < tk) on GpSimd — off
                        # both the PE and the busy Act/DVE engines
                        for lo in (cs, TQ):
                            nc.gpsimd.tensor_tensor(
                                e[:, lo : lo + P],
                                e[:, lo : lo + P],
                                m01_sb[:],
                                MUL,
                            )
                    for par in (0, 1):
                        lo = cs if par == 0 else TQ
                        vlo = g * VW + (HD + 1) * par
                        nc.tensor.matmul(
                            avs[par][:, cs:],
                            v_sb[:, tkc, vlo : vlo + P],
                            e[:, lo : lo + TQ - cs],
                            start=(tkc == 0),
                            stop=(tkc == hi - 1),
                        )

                pend = qk_emit(0)
                drip()
                for tkc in range(1, hi):
                    cur = qk_emit(tkc)
                    drip()
                    ea_emit(*pend, tkc - 1)
                    drip()
                    pend = cur
                ea_emit(*pend, hi - 1)
                drip()
                # evacuate out_aug to SBUF right away in bf16 (frees the
                # psum banks fast) and gather the two denominator rows to
                # [128, 8] via DMA so the reciprocal runs on all lanes.
                asb = rrp.tile([65, 2, TQ], BF16, tag="avsb", name="asb")
                for par in (0, 1):
                    nc.vector.tensor_scalar_add(
                        asb[:, par, :], avs[par][0:65, :], 0.0
                    )
                rd = rrp.tile([P, 8], BF16, tag="rd", name="rd")
                nc.sync.dma_start(rd[:], asb[64:65, :, :])

                rrs = []

                def stage1():
                    rd2 = rrp.tile([P, 8], BF16, tag="rd2", name="rd2")
                    with nc.allow_low_precision(
                        reason="softmax denom reciprocal in bf16; rel err "
                        "~4e-3 well inside the 2e-2 budget"
                    ):
                        nc.vector.reciprocal(rd2[:], rd[:])
                    rro = rrp.tile([1, 2, TQ], BF16, tag="rro", name="rro")
                    nc.sync.dma_start(rro[0:1, :, :], rd2[:])
                    for par in (0, 1):
                        rr = rrp.tile([64, TQ], BF16, tag=f"rr{par}", name="rr")
                        nc.sync.dma_start(
                            rr[:],
                            rro[0:1, par, None, :].to_broadcast([1, 64, TQ]),
                        )
                        rrs.append(rr)

                def stage2():
                    nc.vector.tensor_tensor(
                        rawT[0:64, g, ts(t2, TQ)],
                        asb[0:64, 0, :],
                        rrs[0][:],
                        MUL,
                    )
                    tmp = rrp.tile([64, TQ], BF16, tag="otmp", name="tmp")
                    nc.vector.tensor_tensor(
                        tmp[:], asb[0:64, 1, :], rrs[1][:], MUL
                    )
                    nc.sync.dma_start(rawT[64:128, g, ts(t2, TQ)], tmp[:])

                return stage1, stage2

            def load_wpt(ec):
                wpt = wpp.tile([P, CC, P], BF16, tag="wp", name="wpt")
                nc.sync.dma_start(wpt[:], wp_r[:, :, ts(ec, P)])
                return wpt

            # The projection is split per output half into a 5-chunk
            # partial plus a single final matmul on the last head pair, so
            # no proj work ever waits on the most recent divide chain.
            def proj_partial(t2, ec, wpt):
                ps = psB.tile([P, TQ], F32, tag="mm", name="pspa")
                for jc in range(CC - 1):
                    nc.tensor.matmul(
                        ps[:],
                        wpt[:, jc, :],
                        rawT[:, jc, ts(t2, TQ)],
                        start=(jc == 0),
                        stop=(jc == CC - 2),
                    )
                yta = ytap.tile([P, TQ], F32, tag="yta", name="yta")
                nc.vector.tensor_scalar_add(yta[:], ps[:], 0.0)
                return yta

            def proj_final(t2, ec, wpt, yta):
                ps = psB.tile([P, TQ], F32, tag="mm", name="pspb")
                nc.tensor.matmul(
                    ps[:],
                    wpt[:, CC - 1, :],
                    rawT[:, CC - 1, ts(t2, TQ)],
                    start=True,
                    stop=True,
                )
                yt = ytp.tile([P, TQ], F32, tag="yt", name="yt")
                nc.vector.scalar_tensor_tensor(
                    yt[:], ps[:], bp_sb[:, ec : ec + 1], yta[:], ADD, ADD
                )
                nc.sync.dma_start(yT_r[:, ec, ts(t2, TQ)], yt[:])

            # ---- filler unit factories (append to fillq) --------------
            def queue_v(tc_i):
                for jn in range(C // JV):
                    box = {}

                    def mm(cc, jn=jn, box=box, tc_i=tc_i):
                        if cc == 0:
                            box["ps"] = psB.tile(
                                [P, TQ], F32, tag="mm", name="psv"
                            )
                        nc.tensor.matmul(
                            box["ps"][:, :JV],
                            xts[cc][:, ts(tc_i, P)],
                            wv_sb[:, cc, ts(jn, JV)],
                            start=(cc == 0),
                            stop=(cc == CC - 1),
                        )

                    def ev(jn=jn, box=box, tc_i=tc_i):
                        g0 = jn * (JV // P)
                        srcv = box["ps"][:, :JV].rearrange(
                            "p (g h d) -> p g h d", h=2, d=HD
                        )
                        dstv = v4[:, tc_i, g0 : g0 + 3, :].rearrange(
                            "p g (h w) -> p g h w", h=2
                        )[:, :, :, :HD]
                        nc.scalar.activation(dstv, srcv, AF.Identity)

                    for cc in range(CC):
                        fillq.append(lambda cc=cc, mm=mm: mm(cc))
                    fillq.append(ev)

            def queue_proj_partial(t2, ec, wpt, store):
                box = {}

                def mm(jc, box=box):
                    if jc == 0:
                        box["ps"] = psB.tile([P, TQ], F32, tag="mm", name="pspa")
                    nc.tensor.matmul(
                        box["ps"][:],
                        wpt[:, jc, :],
                        rawT[:, jc, ts(t2, TQ)],
                        start=(jc == 0),
                        stop=(jc == CC - 2),
                    )

                def ev(box=box):
                    yta = ytap.tile([P, TQ], F32, tag="yta", name="yta")
                    nc.vector.tensor_scalar_add(yta[:], box["ps"][:], 0.0)
                    store[ec] = yta

                for jc in range(CC - 1):
                    fillq.append(lambda jc=jc, mm=mm: mm(jc))
                fillq.append(ev)

            def queue_proj_final(t2, ec, wpt, store):
                def f():
                    proj_final(t2, ec, wpt, store[ec])

                fillq.append(f)

            # ---- emission schedule ------------------------------------
            # backbone: all six t2=0 attention blocks, then all six t2=1
            # blocks.  The t2=1 phase is Act(exp)-bound, so the remaining
            # GEMM work (t2=0 projections) drips into its PE bubbles.
            # `fins` holds the pending divide stages: after emitting block
            # k, run stage1 of block k-1 and stage2 of block k-2.
            fins = []

            def push_fins(s=None):
                if len(fins) >= 1 and fins[-1][0]:
                    fins[-1][0]()
                    fins[-1][0] = None
                if len(fins) >= 2 and fins[-2][1]:
                    fins[-2][1]()
                    fins[-2][1] = None
                if s is not None:
                    fins.append(list(s))

            wts = (load_wt(0), load_wt(G))
            nc.sync.dma_start(wv_sb[:], wv_r)
            nxt = (load_wt(1), load_wt(G + 1))
            qkt_chunk(0, wts[0])
            qkt_chunk(G, wts[1])
            v_part(range(0, 4))
            qkt_chunk(1, nxt[0])
            qkt_chunk(G + 1, nxt[1])
            nxt = (load_wt(2), load_wt(G + 2))
            for tc_i in range(4, 8):
                queue_v(tc_i)  # drips into the t2=0 blocks
            ytas0 = [None] * EC
            ytas1 = [None] * EC
            wpts = []
            for g in range(G):
                if g >= 2:
                    wts = nxt
                    qkt_chunk(g, wts[0])
                    qkt_chunk(G + g, wts[1])
                    if g < G - 1:
                        nxt = (load_wt(g + 1), load_wt(G + g + 1))
                    if g >= 3:
                        wpts.append(load_wpt(2 * g - 6))
                        wpts.append(load_wpt(2 * g - 5))
                push_fins(attn_block(g, 0))
            flushq()  # any v units not yet dripped (needed by attn(0,1))
            for g in range(G):
                push_fins(attn_block(g, 1))
                if g == 0:
                    # stage2 of (4,0) was just emitted: rawT[:, 0:5, t2=0]
                    # is complete; drip the t2=0 partial projections
                    for ec in range(EC):
                        queue_proj_partial(0, ec, wpts[ec], ytas0)
                elif g == 1:
                    # stage2 of (5,0) emitted: the finals can follow
                    for ec in range(EC):
                        queue_proj_final(0, ec, wpts[ec], ytas0)
            # drain: leftover fillers, then the last two divide chains
            # with the t2=1 projections interleaved
            flushq()
            fins[-1][0]()
            fins[-2][1]()
            for ec in range(EC):
                ytas1[ec] = proj_partial(1, ec, wpts[ec])
            fins[-1][1]()
            for ec in range(EC):
                proj_final(1, ec, wpts[ec], ytas1[ec])

    nc.compile()
    _CACHE["nc"] = nc
    return nc


def make_in_maps(x, w_attn, b_attn, w_proj, b_proj):
    x = np.ascontiguousarray(np.asarray(x, dtype=np.float32))
    w_attn = np.ascontiguousarray(np.asarray(w_attn, dtype=np.float32))
    b_attn = np.ascontiguousarray(np.asarray(b_attn, dtype=np.float32))
    w_proj = np.ascontiguousarray(np.asarray(w_proj, dtype=np.float32))
    b_proj = np.ascontiguousarray(np.asarray(b_proj, dtype=np.float32))

    bf = ml_dtypes.bfloat16
    wqk = w_attn[:, : 2 * C].astype(bf)
    wv = w_attn[:, 2 * C :].astype(bf)
    wp_b = w_proj.astype(bf)
    bqk = np.ascontiguousarray(b_attn[: 2 * C].reshape(JQK, P).T)
    # the v bias folds into the projection bias: y = Wp.T (raw + bv) + bp
    bv = b_attn[2 * C :].astype(np.float64)
    bp_eff = (w_proj.astype(np.float64).T @ bv + b_proj).astype(np.float32)
    bp = np.ascontiguousarray(bp_eff.reshape(EC, P).T)
    # m01[p, j] = 1 where j >= p (keep), 0 above-diagonal columns
    m01 = (np.arange(P)[None, :] >= np.arange(P)[:, None]).astype(bf)

    shared = {
        "wqk": wqk,
        "wv": wv,
        "wp": wp_b,
        "bqk": bqk,
        "bp": bp,
        "m01": m01,
    }
    return [
        {"xT": np.ascontiguousarray(x[b].T).astype(bf), **shared}
        for b in range(NCORES)
    ]


def kernel(**inputs):
    nc = _build()
    in_maps = make_in_maps(
        inputs["x"],
        inputs["w_attn"],
        inputs["b_attn"],
        inputs["w_proj"],
        inputs["b_proj"],
    )
    res = run_bass_kernel_spmd(nc, in_maps, list(range(NCORES)))
    out = np.stack(
        [np.ascontiguousarray(res.results[b]["yT"].T) for b in range(NCORES)]
    )
    return out.astype(np.float32)
